# revision 12
# baseline (speedup 1.0000x reference)
"""IrregularRNN (exact LTC cell) Trainium2 Bass kernel.

Strategy: tensor-parallel split of the 2U=2048 pre-activation columns
across 8 cores. Core k computes pre columns {f: [k*128,(k+1)*128),
a: [U+k*128, U+(k+1)*128)} for the FULL batch B=128 (full PE
utilization), updates h columns [k*128,(k+1)*128), transposes its
h'-slice on the PE, and AllGathers the transposed slices so every core
has the full h^T (as 8 ready-to-use lhsT K-chunks) for the next step's
h @ Wh matmul.  The x_t @ Wx part + bias only depend on inputs, so
those matmuls are issued ahead and hide inside the AllGather wait.

All layout transforms (transposes, weight slicing, broadcast of tau)
are done host-side in numpy; the device loop is 256 fully-unrolled
steps.
"""

import sys

sys.path.insert(0, "/opt/trn_rl_repo")

import numpy as np

B, T, D, U = 128, 256, 256, 1024
NC = 8
SL = U // NC          # h columns per core (128)
PW = 2 * SL           # pre-activation columns per core (256)
DK = D // 128         # K-chunks for x part (2)
UK = U // 128         # K-chunks for h part (8)

_CACHE: dict = {}


def _build(n_steps: int, use_collective: bool = True, repeat: int = 1,
           use_fp32r: bool = False):
    """Build + bacc-compile the SPMD Bass module for n_steps timesteps.

    repeat>1 is a timing-only mode: the T-loop body runs repeat times over
    the same inputs/outputs (numerically wrong; isolates on-device exec
    time from harness data-shipping via wall(2x) - wall(1x)).

    use_fp32r: run the PE matmuls in float32r (replicated fp32) — 4x the
    fp32 streaming rate at N>=256; bit-identical input layout."""
    import concourse.bacc as bacc
    import concourse.tile as tile
    from concourse import mybir

    AF = mybir.ActivationFunctionType
    f32 = mybir.dt.float32
    fmm = mybir.dt.float32r if use_fp32r else f32

    def mm(ap):
        return ap.bitcast(fmm) if use_fp32r else ap

    nc = bacc.Bacc(
        "TRN2",
        target_bir_lowering=False,
        debug=False,
        enable_asserts=False,
        num_devices=NC,
    )

    # --- kernel I/O ---------------------------------------------------
    xT = nc.dram_tensor("xT", [n_steps, DK, 128, B], f32, kind="ExternalInput")
    wx_sl = nc.dram_tensor("wx_sl", [DK, 128, PW], f32, kind="ExternalInput")
    wh_sl = nc.dram_tensor("wh_sl", [UK, 128, PW], f32, kind="ExternalInput")
    b_sl = nc.dram_tensor("b_sl", [1, PW], f32, kind="ExternalInput")
    tau_b = nc.dram_tensor("tau_b", [B, SL], f32, kind="ExternalInput")
    ndt = nc.dram_tensor("ndt", [B, n_steps], f32, kind="ExternalInput")
    h0T = nc.dram_tensor("h0T", [UK, 128, B], f32, kind="ExternalInput")
    h0_sl = nc.dram_tensor("h0_sl", [B, SL], f32, kind="ExternalInput")
    ones = nc.dram_tensor("ones", [1, 128], f32, kind="ExternalInput")
    ident = nc.dram_tensor("ident", [128, 128], f32, kind="ExternalInput")
    ys_sl = nc.dram_tensor("ys_sl", [n_steps, B, SL], f32, kind="ExternalOutput")

    RG = [list(range(NC))]

    with tile.TileContext(nc) as tc:
        with (
            tc.tile_pool(name="const", bufs=1) as cpool,
            tc.tile_pool(name="xin", bufs=6) as xpool,
            tc.tile_pool(name="hT", bufs=2) as hTpool,
            tc.tile_pool(name="act", bufs=3) as apool,
            tc.tile_pool(name="hnew", bufs=3) as hpool,
            tc.tile_pool(name="pre", bufs=2, space="PSUM") as prepool,
            tc.tile_pool(name="trp", bufs=2, space="PSUM") as trpool,
            tc.tile_pool(name="agio", bufs=2, space="DRAM") as dpool,
        ):
            # --- constants, loaded once -------------------------------
            wx_sb = cpool.tile([128, DK, PW], f32, name="wx_sb")
            nc.sync.dma_start(out=wx_sb[:], in_=wx_sl.ap().rearrange("c p n -> p c n"))
            wh_sb = cpool.tile([128, UK, PW], f32, name="wh_sb")
            nc.sync.dma_start(out=wh_sb[:], in_=wh_sl.ap().rearrange("c p n -> p c n"))
            b_sb = cpool.tile([1, PW], f32, name="b_sb")
            nc.sync.dma_start(out=b_sb[:], in_=b_sl[:])
            ones_sb = cpool.tile([1, 128], f32, name="ones_sb")
            nc.sync.dma_start(out=ones_sb[:], in_=ones[:])
            tau_sb = cpool.tile([B, SL], f32, name="tau_sb")
            nc.sync.dma_start(out=tau_sb[:], in_=tau_b[:])
            ndt_sb = cpool.tile([B, n_steps], f32, name="ndt_sb")
            nc.sync.dma_start(out=ndt_sb[:], in_=ndt[:])
            ident_sb = cpool.tile([128, 128], f32, name="ident_sb")
            nc.sync.dma_start(out=ident_sb[:], in_=ident[:])

            # initial state
            h_prev = hpool.tile([B, SL], f32, name="h_new")
            nc.sync.dma_start(out=h_prev[:], in_=h0_sl[:])
            hT0 = hTpool.tile([128, UK, B], f32, name="hTall")
            nc.sync.dma_start(out=hT0[:], in_=h0T.ap().rearrange("c p b -> p c b"))
            hT_cur = [hT0[:, j, :] for j in range(UK)]

            # --- the recurrence ---------------------------------------
            for tv in range(n_steps * repeat):
                t = tv % n_steps
                # x_t^T chunks: [d-chunk partitions, batch free]
                xt = xpool.tile([128, DK, B], f32, name="xt")
                nc.sync.dma_start(out=xt[:], in_=xT[t].rearrange("c p b -> p c b"))

                pre = prepool.tile([B, PW], f32, name="pre")
                # bias + x part: no dependency on h -> runs during the
                # previous step's AllGather wait.
                nc.tensor.matmul(pre[:], mm(ones_sb[:]), mm(b_sb[:]), start=True, stop=False)
                for c in range(DK):
                    nc.tensor.matmul(
                        pre[:], mm(xt[:, c, :]), mm(wx_sb[:, c, :]),
                        start=False, stop=False,
                    )
                # h part
                for j in range(UK):
                    nc.tensor.matmul(
                        pre[:],
                        mm(hT_cur[j]),
                        mm(wh_sb[:, j, :]),
                        start=False,
                        stop=(j == UK - 1),
                    )

                # sigmoid via tanh keeps Tanh+Exp in ONE activation table
                # set (exp_and_others): no per-step ACT_TABLE_LOAD thrash.
                # sigmoid(x) = 0.5*tanh(x/2) + 0.5; host ships tau+0.5, so
                # g = (tau+0.5) + 0.5*tanh(pre_f/2) = tau + sigmoid(pre_f).
                f = apool.tile([B, SL], f32, name="f")
                nc.scalar.activation(f[:], pre[:, 0:SL], AF.Tanh, scale=0.5)
                a = apool.tile([B, SL], f32, name="a")
                nc.scalar.activation(a[:], pre[:, SL:PW], AF.Tanh)
                g = apool.tile([B, SL], f32, name="g")
                nc.vector.scalar_tensor_tensor(
                    g[:], f[:], 0.5, tau_sb[:],
                    op0=mybir.AluOpType.mult, op1=mybir.AluOpType.add,
                )
                dcy = apool.tile([B, SL], f32, name="dcy")
                nc.scalar.activation(
                    dcy[:], g[:], AF.Exp, scale=ndt_sb[:, t : t + 1]
                )
                hma = apool.tile([B, SL], f32, name="hma")
                nc.vector.tensor_sub(hma[:], h_prev[:], a[:])
                hd = apool.tile([B, SL], f32, name="hd")
                nc.vector.tensor_mul(hd[:], hma[:], dcy[:])
                h_new = hpool.tile([B, SL], f32, name="h_new")
                nc.vector.tensor_add(h_new[:], hd[:], a[:])

                nc.sync.dma_start(out=ys_sl[t], in_=h_new[:])

                if tv == n_steps * repeat - 1:
                    h_prev = h_new
                    break

                # h'^T slice for the next step's matmul
                trp = trpool.tile([128, B], f32, name="trp")
                nc.tensor.transpose(trp[:], h_new[:], ident_sb[:])
                trs = apool.tile([128, B], f32, name="trs")
                nc.vector.tensor_copy(trs[:], trp[:])
                ag_in = dpool.tile([128, B], f32, name="ag_in")
                nc.sync.dma_start(out=ag_in[:], in_=trs[:])
                if use_collective:
                    ag_out = dpool.tile(
                        [UK * 128, B], f32, name="ag_out", addr_space="Shared"
                    )
                    nc.gpsimd.collective_compute(
                        "AllGather",
                        mybir.AluOpType.bypass,
                        replica_groups=RG,
                        ins=[ag_in[:].opt()],
                        outs=[ag_out[:].opt()],
                    )
                hTall = hTpool.tile([128, UK, B], f32, name="hTall")
                if use_collective:
                    # one strided DMA gathers all 8 K-chunks: [(c p) b -> p c b]
                    nc.sync.dma_start(
                        out=hTall[:],
                        in_=ag_out.rearrange("(c p) b -> p c b", p=128),
                    )
                else:
                    # timing-only bisect variant: local slice in place of
                    # the gathered one (numerically wrong on purpose)
                    for j in range(UK):
                        nc.sync.dma_start(out=hTall[:, j, :], in_=ag_in[:])
                hT_cur = [hTall[:, j, :] for j in range(UK)]
                h_prev = h_new

    nc.compile()
    return nc


def _prep_inputs(features, time_steps, Wx, Wh, b, w_tau, h0, n_steps):
    """Host-side sharding + layout transforms -> per-core in_maps."""
    f32 = np.float32
    features = np.asarray(features, dtype=f32)
    time_steps = np.asarray(time_steps, dtype=f32)
    Wx = np.asarray(Wx, dtype=f32)
    Wh = np.asarray(Wh, dtype=f32)
    b = np.asarray(b, dtype=f32)
    w_tau = np.asarray(w_tau, dtype=f32)
    h0 = np.asarray(h0, dtype=f32)

    # softplus(w_tau) + 0.5 (the 0.5 folds the tanh-form sigmoid offset)
    tau = (np.log1p(np.exp(w_tau)) + 0.5).astype(f32)

    xT = np.ascontiguousarray(features.transpose(1, 2, 0)).reshape(n_steps, DK, 128, B)
    ndt = np.ascontiguousarray(-time_steps)                      # [B, T]
    h0T = np.ascontiguousarray(h0.T).reshape(UK, 128, B)
    ones = np.ones((1, 128), dtype=f32)
    ident = np.eye(128, dtype=f32)

    in_maps = []
    for k in range(NC):
        cols = np.concatenate(
            [np.arange(k * SL, (k + 1) * SL), U + np.arange(k * SL, (k + 1) * SL)]
        )
        in_maps.append(
            {
                "xT": xT,
                "wx_sl": np.ascontiguousarray(Wx[:, cols]).reshape(DK, 128, PW),
                "wh_sl": np.ascontiguousarray(Wh[:, cols]).reshape(UK, 128, PW),
                "b_sl": np.ascontiguousarray(b[cols]).reshape(1, PW),
                "tau_b": np.ascontiguousarray(
                    np.broadcast_to(tau[k * SL : (k + 1) * SL], (B, SL))
                ),
                "ndt": ndt,
                "h0T": h0T,
                "h0_sl": np.ascontiguousarray(h0[:, k * SL : (k + 1) * SL]),
                "ones": ones,
                "ident": ident,
            }
        )
    return in_maps


def _assemble(results):
    """[T, B, SL] slices per core -> [B, T, U] full output."""
    ys = np.concatenate([r["ys_sl"] for r in results], axis=2)  # [T, B, U]
    return np.ascontiguousarray(ys.transpose(1, 0, 2))


def kernel(features, time_steps, Wx, Wh, b, w_tau, h0, _trace=False):
    from concourse import bass_utils

    n_steps = features.shape[1]
    if n_steps not in _CACHE:
        _CACHE[n_steps] = _build(n_steps)
    nc = _CACHE[n_steps]

    in_maps = _prep_inputs(features, time_steps, Wx, Wh, b, w_tau, h0, n_steps)
    try:
        res = bass_utils.run_bass_kernel_spmd(
            nc, in_maps, core_ids=list(range(NC)), trace=_trace
        )
    except ModuleNotFoundError:
        # no NTFF profiling hook in this container — run untraced
        res = bass_utils.run_bass_kernel_spmd(
            nc, in_maps, core_ids=list(range(NC)), trace=False
        )
    out = _assemble(res.results)
    if _trace:
        return out, res
    return out


if __name__ == "__main__":
    # smoke test with random data
    rng = np.random.default_rng(0)
    feats = rng.standard_normal((B, T, D), dtype=np.float32)
    ts = rng.random((B, T), dtype=np.float32)
    Wx = rng.standard_normal((D, 2 * U), dtype=np.float32) / np.sqrt(D)
    Wh = rng.standard_normal((U, 2 * U), dtype=np.float32) / np.sqrt(U)
    b = np.zeros((2 * U,), dtype=np.float32)
    w_tau = rng.random((U,), dtype=np.float32)
    h0 = np.zeros((B, U), dtype=np.float32)
    out = kernel(feats, ts, Wx, Wh, b, w_tau, h0)
    print("output", out.shape, out.dtype)



# revision 16
# speedup vs baseline: 1.0214x; 1.0214x over previous
"""IrregularRNN (exact LTC cell) Trainium2 Bass kernel.

Strategy: tensor-parallel split of the 2U=2048 pre-activation columns
across 8 cores. Core k computes pre columns {f: [k*128,(k+1)*128),
a: [U+k*128, U+(k+1)*128)} for the FULL batch B=128 (full PE
utilization), updates h columns [k*128,(k+1)*128), transposes its
h'-slice on the PE, and AllGathers the transposed slices so every core
has the full h^T (as 8 ready-to-use lhsT K-chunks) for the next step's
h @ Wh matmul.  The x_t @ Wx part + bias only depend on inputs, so
those matmuls are issued ahead and hide inside the AllGather wait.

All layout transforms (transposes, weight slicing, broadcast of tau)
are done host-side in numpy; the device loop is 256 fully-unrolled
steps.
"""

import sys

sys.path.insert(0, "/opt/trn_rl_repo")

import numpy as np

B, T, D, U = 128, 256, 256, 1024
NC = 8
SL = U // NC          # h columns per core (128)
PW = 2 * SL           # pre-activation columns per core (256)
DK = D // 128         # K-chunks for x part (2)
UK = U // 128         # K-chunks for h part (8)

_CACHE: dict = {}


def _build(n_steps: int, use_collective: bool = True, repeat: int = 1,
           use_fp32r: bool = False):
    """Build + bacc-compile the SPMD Bass module for n_steps timesteps.

    repeat>1 is a timing-only mode: the T-loop body runs repeat times over
    the same inputs/outputs (numerically wrong; isolates on-device exec
    time from harness data-shipping via wall(2x) - wall(1x)).

    use_fp32r: run the PE matmuls in float32r (replicated fp32) — 4x the
    fp32 streaming rate at N>=256; bit-identical input layout."""
    import concourse.bacc as bacc
    import concourse.tile as tile
    from concourse import mybir

    AF = mybir.ActivationFunctionType
    f32 = mybir.dt.float32
    fmm = mybir.dt.float32r if use_fp32r else f32

    def mm(ap):
        return ap.bitcast(fmm) if use_fp32r else ap

    nc = bacc.Bacc(
        "TRN2",
        target_bir_lowering=False,
        debug=False,
        enable_asserts=False,
        num_devices=NC,
    )

    # --- kernel I/O ---------------------------------------------------
    xT = nc.dram_tensor("xT", [n_steps, DK, 128, B], f32, kind="ExternalInput")
    wx_sl = nc.dram_tensor("wx_sl", [DK, 128, PW], f32, kind="ExternalInput")
    wh_sl = nc.dram_tensor("wh_sl", [UK, 128, PW], f32, kind="ExternalInput")
    b_sl = nc.dram_tensor("b_sl", [1, PW], f32, kind="ExternalInput")
    tau_b = nc.dram_tensor("tau_b", [B, SL], f32, kind="ExternalInput")
    ndt = nc.dram_tensor("ndt", [B, n_steps], f32, kind="ExternalInput")
    h0T = nc.dram_tensor("h0T", [UK, 128, B], f32, kind="ExternalInput")
    h0_sl = nc.dram_tensor("h0_sl", [B, SL], f32, kind="ExternalInput")
    ones = nc.dram_tensor("ones", [1, 128], f32, kind="ExternalInput")
    ident = nc.dram_tensor("ident", [128, 128], f32, kind="ExternalInput")
    ys_sl = nc.dram_tensor("ys_sl", [n_steps, B, SL], f32, kind="ExternalOutput")

    RG = [list(range(NC))]

    with tile.TileContext(nc) as tc:
        with (
            tc.tile_pool(name="const", bufs=1) as cpool,
            tc.tile_pool(name="xin", bufs=6) as xpool,
            tc.tile_pool(name="hT", bufs=2) as hTpool,
            tc.tile_pool(name="act", bufs=3) as apool,
            tc.tile_pool(name="hnew", bufs=3) as hpool,
            tc.tile_pool(name="pre", bufs=2, space="PSUM") as prepool,
            tc.tile_pool(name="trp", bufs=2, space="PSUM") as trpool,
            tc.tile_pool(name="agio", bufs=2, space="DRAM") as dpool,
        ):
            # --- constants, loaded once -------------------------------
            wx_sb = cpool.tile([128, DK, PW], f32, name="wx_sb")
            nc.sync.dma_start(out=wx_sb[:], in_=wx_sl.ap().rearrange("c p n -> p c n"))
            wh_sb = cpool.tile([128, UK, PW], f32, name="wh_sb")
            nc.sync.dma_start(out=wh_sb[:], in_=wh_sl.ap().rearrange("c p n -> p c n"))
            b_sb = cpool.tile([1, PW], f32, name="b_sb")
            nc.sync.dma_start(out=b_sb[:], in_=b_sl[:])
            ones_sb = cpool.tile([1, 128], f32, name="ones_sb")
            nc.sync.dma_start(out=ones_sb[:], in_=ones[:])
            tau_sb = cpool.tile([B, SL], f32, name="tau_sb")
            nc.sync.dma_start(out=tau_sb[:], in_=tau_b[:])
            ndt_sb = cpool.tile([B, n_steps], f32, name="ndt_sb")
            nc.sync.dma_start(out=ndt_sb[:], in_=ndt[:])
            ident_sb = cpool.tile([128, 128], f32, name="ident_sb")
            nc.sync.dma_start(out=ident_sb[:], in_=ident[:])

            # initial state
            h_prev = hpool.tile([B, SL], f32, name="h_new")
            nc.sync.dma_start(out=h_prev[:], in_=h0_sl[:])
            hT0 = hTpool.tile([128, UK, B], f32, name="hTall")
            nc.sync.dma_start(out=hT0[:], in_=h0T.ap().rearrange("c p b -> p c b"))
            hT_cur = [hT0[:, j, :] for j in range(UK)]

            # --- the recurrence ---------------------------------------
            for tv in range(n_steps * repeat):
                t = tv % n_steps
                # x_t^T chunks: [d-chunk partitions, batch free]
                xt = xpool.tile([128, DK, B], f32, name="xt")
                nc.sync.dma_start(out=xt[:], in_=xT[t].rearrange("c p b -> p c b"))

                # split PSUM accumulation into f-half and a-half: pre_f stops
                # 8 matmuls earlier, so the f-chain (tanh -> g -> exp) hides
                # under the a-half h-matmuls.
                pre_f = prepool.tile([B, SL], f32, name="pre_f")
                pre_a = prepool.tile([B, SL], f32, name="pre_a")
                # bias + x part: no dependency on h -> runs during the
                # previous step's AllGather wait.
                nc.tensor.matmul(pre_f[:], mm(ones_sb[:]), mm(b_sb[:, 0:SL]),
                                 start=True, stop=False)
                nc.tensor.matmul(pre_a[:], mm(ones_sb[:]), mm(b_sb[:, SL:PW]),
                                 start=True, stop=False)
                for c in range(DK):
                    nc.tensor.matmul(
                        pre_f[:], mm(xt[:, c, :]), mm(wx_sb[:, c, 0:SL]),
                        start=False, stop=False,
                    )
                    nc.tensor.matmul(
                        pre_a[:], mm(xt[:, c, :]), mm(wx_sb[:, c, SL:PW]),
                        start=False, stop=False,
                    )
                # h part: all f-half chunks first (early stop), then a-half
                for j in range(UK):
                    nc.tensor.matmul(
                        pre_f[:],
                        mm(hT_cur[j]),
                        mm(wh_sb[:, j, 0:SL]),
                        start=False,
                        stop=(j == UK - 1),
                    )
                for j in range(UK):
                    nc.tensor.matmul(
                        pre_a[:],
                        mm(hT_cur[j]),
                        mm(wh_sb[:, j, SL:PW]),
                        start=False,
                        stop=(j == UK - 1),
                    )

                # sigmoid via tanh keeps Tanh+Exp in ONE activation table
                # set (exp_and_others): no per-step ACT_TABLE_LOAD thrash.
                # sigmoid(x) = 0.5*tanh(x/2) + 0.5; host ships tau+0.5, so
                # g = (tau+0.5) + 0.5*tanh(pre_f/2) = tau + sigmoid(pre_f).
                f = apool.tile([B, SL], f32, name="f")
                nc.scalar.activation(f[:], pre_f[:], AF.Tanh, scale=0.5)
                a = apool.tile([B, SL], f32, name="a")
                nc.scalar.activation(a[:], pre_a[:], AF.Tanh)
                g = apool.tile([B, SL], f32, name="g")
                nc.vector.scalar_tensor_tensor(
                    g[:], f[:], 0.5, tau_sb[:],
                    op0=mybir.AluOpType.mult, op1=mybir.AluOpType.add,
                )
                dcy = apool.tile([B, SL], f32, name="dcy")
                nc.scalar.activation(
                    dcy[:], g[:], AF.Exp, scale=ndt_sb[:, t : t + 1]
                )
                hma = apool.tile([B, SL], f32, name="hma")
                nc.vector.tensor_sub(hma[:], h_prev[:], a[:])
                hd = apool.tile([B, SL], f32, name="hd")
                nc.vector.tensor_mul(hd[:], hma[:], dcy[:])
                h_new = hpool.tile([B, SL], f32, name="h_new")
                nc.vector.tensor_add(h_new[:], hd[:], a[:])

                nc.sync.dma_start(out=ys_sl[t], in_=h_new[:])

                if tv == n_steps * repeat - 1:
                    h_prev = h_new
                    break

                # h'^T slice for the next step's matmul
                trp = trpool.tile([128, B], f32, name="trp")
                nc.tensor.transpose(trp[:], h_new[:], ident_sb[:])
                trs = apool.tile([128, B], f32, name="trs")
                nc.vector.tensor_copy(trs[:], trp[:])
                ag_in = dpool.tile([128, B], f32, name="ag_in")
                nc.sync.dma_start(out=ag_in[:], in_=trs[:])
                if use_collective:
                    ag_out = dpool.tile(
                        [UK * 128, B], f32, name="ag_out", addr_space="Shared"
                    )
                    nc.gpsimd.collective_compute(
                        "AllGather",
                        mybir.AluOpType.bypass,
                        replica_groups=RG,
                        ins=[ag_in[:].opt()],
                        outs=[ag_out[:].opt()],
                    )
                hTall = hTpool.tile([128, UK, B], f32, name="hTall")
                if use_collective:
                    # one strided DMA gathers all 8 K-chunks: [(c p) b -> p c b]
                    nc.sync.dma_start(
                        out=hTall[:],
                        in_=ag_out.rearrange("(c p) b -> p c b", p=128),
                    )
                else:
                    # timing-only bisect variant: local slice in place of
                    # the gathered one (numerically wrong on purpose)
                    for j in range(UK):
                        nc.sync.dma_start(out=hTall[:, j, :], in_=ag_in[:])
                hT_cur = [hTall[:, j, :] for j in range(UK)]
                h_prev = h_new

    nc.compile()
    return nc


def _prep_inputs(features, time_steps, Wx, Wh, b, w_tau, h0, n_steps):
    """Host-side sharding + layout transforms -> per-core in_maps."""
    f32 = np.float32
    features = np.asarray(features, dtype=f32)
    time_steps = np.asarray(time_steps, dtype=f32)
    Wx = np.asarray(Wx, dtype=f32)
    Wh = np.asarray(Wh, dtype=f32)
    b = np.asarray(b, dtype=f32)
    w_tau = np.asarray(w_tau, dtype=f32)
    h0 = np.asarray(h0, dtype=f32)

    # softplus(w_tau) + 0.5 (the 0.5 folds the tanh-form sigmoid offset)
    tau = (np.log1p(np.exp(w_tau)) + 0.5).astype(f32)

    xT = np.ascontiguousarray(features.transpose(1, 2, 0)).reshape(n_steps, DK, 128, B)
    ndt = np.ascontiguousarray(-time_steps)                      # [B, T]
    h0T = np.ascontiguousarray(h0.T).reshape(UK, 128, B)
    ones = np.ones((1, 128), dtype=f32)
    ident = np.eye(128, dtype=f32)

    in_maps = []
    for k in range(NC):
        cols = np.concatenate(
            [np.arange(k * SL, (k + 1) * SL), U + np.arange(k * SL, (k + 1) * SL)]
        )
        in_maps.append(
            {
                "xT": xT,
                "wx_sl": np.ascontiguousarray(Wx[:, cols]).reshape(DK, 128, PW),
                "wh_sl": np.ascontiguousarray(Wh[:, cols]).reshape(UK, 128, PW),
                "b_sl": np.ascontiguousarray(b[cols]).reshape(1, PW),
                "tau_b": np.ascontiguousarray(
                    np.broadcast_to(tau[k * SL : (k + 1) * SL], (B, SL))
                ),
                "ndt": ndt,
                "h0T": h0T,
                "h0_sl": np.ascontiguousarray(h0[:, k * SL : (k + 1) * SL]),
                "ones": ones,
                "ident": ident,
            }
        )
    return in_maps


def _assemble(results):
    """[T, B, SL] slices per core -> [B, T, U] full output."""
    ys = np.concatenate([r["ys_sl"] for r in results], axis=2)  # [T, B, U]
    return np.ascontiguousarray(ys.transpose(1, 0, 2))


def kernel(features, time_steps, Wx, Wh, b, w_tau, h0, _trace=False):
    from concourse import bass_utils

    n_steps = features.shape[1]
    if n_steps not in _CACHE:
        _CACHE[n_steps] = _build(n_steps)
    nc = _CACHE[n_steps]

    in_maps = _prep_inputs(features, time_steps, Wx, Wh, b, w_tau, h0, n_steps)
    try:
        res = bass_utils.run_bass_kernel_spmd(
            nc, in_maps, core_ids=list(range(NC)), trace=_trace
        )
    except ModuleNotFoundError:
        # no NTFF profiling hook in this container — run untraced
        res = bass_utils.run_bass_kernel_spmd(
            nc, in_maps, core_ids=list(range(NC)), trace=False
        )
    out = _assemble(res.results)
    if _trace:
        return out, res
    return out


if __name__ == "__main__":
    # smoke test with random data
    rng = np.random.default_rng(0)
    feats = rng.standard_normal((B, T, D), dtype=np.float32)
    ts = rng.random((B, T), dtype=np.float32)
    Wx = rng.standard_normal((D, 2 * U), dtype=np.float32) / np.sqrt(D)
    Wh = rng.standard_normal((U, 2 * U), dtype=np.float32) / np.sqrt(U)
    b = np.zeros((2 * U,), dtype=np.float32)
    w_tau = rng.random((U,), dtype=np.float32)
    h0 = np.zeros((B, U), dtype=np.float32)
    out = kernel(feats, ts, Wx, Wh, b, w_tau, h0)
    print("output", out.shape, out.dtype)



# revision 18
# speedup vs baseline: 1.0259x; 1.0045x over previous
"""IrregularRNN (exact LTC cell) Trainium2 Bass kernel.

Strategy: tensor-parallel split of the 2U=2048 pre-activation columns
across 8 cores. Core k computes pre columns {f: [k*128,(k+1)*128),
a: [U+k*128, U+(k+1)*128)} for the FULL batch B=128 (full PE
utilization), updates h columns [k*128,(k+1)*128), transposes its
h'-slice on the PE, and AllGathers the transposed slices so every core
has the full h^T (as 8 ready-to-use lhsT K-chunks) for the next step's
h @ Wh matmul.  The x_t @ Wx part + bias only depend on inputs, so
those matmuls are issued ahead and hide inside the AllGather wait.

All layout transforms (transposes, weight slicing, broadcast of tau)
are done host-side in numpy; the device loop is 256 fully-unrolled
steps.
"""

import sys

sys.path.insert(0, "/opt/trn_rl_repo")

import numpy as np

B, T, D, U = 128, 256, 256, 1024
NC = 8
SL = U // NC          # h columns per core (128)
PW = 2 * SL           # pre-activation columns per core (256)
DK = D // 128         # K-chunks for x part (2)
UK = U // 128         # K-chunks for h part (8)

_CACHE: dict = {}


def _build(n_steps: int, use_collective: bool = True, repeat: int = 1,
           use_fp32r: bool = False):
    """Build + bacc-compile the SPMD Bass module for n_steps timesteps.

    repeat>1 is a timing-only mode: the T-loop body runs repeat times over
    the same inputs/outputs (numerically wrong; isolates on-device exec
    time from harness data-shipping via wall(2x) - wall(1x)).

    use_fp32r: run the PE matmuls in float32r (replicated fp32) — 4x the
    fp32 streaming rate at N>=256; bit-identical input layout."""
    import concourse.bacc as bacc
    import concourse.tile as tile
    from concourse import mybir

    AF = mybir.ActivationFunctionType
    f32 = mybir.dt.float32
    fmm = mybir.dt.float32r if use_fp32r else f32

    def mm(ap):
        return ap.bitcast(fmm) if use_fp32r else ap

    nc = bacc.Bacc(
        "TRN2",
        target_bir_lowering=False,
        debug=False,
        enable_asserts=False,
        num_devices=NC,
    )

    # --- kernel I/O ---------------------------------------------------
    xT = nc.dram_tensor("xT", [n_steps, DK, 128, B], f32, kind="ExternalInput")
    wx_sl = nc.dram_tensor("wx_sl", [DK, 128, PW], f32, kind="ExternalInput")
    wh_sl = nc.dram_tensor("wh_sl", [UK, 128, PW], f32, kind="ExternalInput")
    b_sl = nc.dram_tensor("b_sl", [1, PW], f32, kind="ExternalInput")
    tau_b = nc.dram_tensor("tau_b", [B, SL], f32, kind="ExternalInput")
    ndt = nc.dram_tensor("ndt", [B, n_steps], f32, kind="ExternalInput")
    h0T = nc.dram_tensor("h0T", [UK, 128, B], f32, kind="ExternalInput")
    h0_sl = nc.dram_tensor("h0_sl", [B, SL], f32, kind="ExternalInput")
    ones = nc.dram_tensor("ones", [1, 128], f32, kind="ExternalInput")
    ident = nc.dram_tensor("ident", [128, 128], f32, kind="ExternalInput")
    ys_sl = nc.dram_tensor("ys_sl", [n_steps, B, SL], f32, kind="ExternalOutput")

    RG = [list(range(NC))]

    with tile.TileContext(nc) as tc:
        with (
            tc.tile_pool(name="const", bufs=1) as cpool,
            tc.tile_pool(name="xin", bufs=6) as xpool,
            tc.tile_pool(name="hT", bufs=2) as hTpool,
            tc.tile_pool(name="act", bufs=3) as apool,
            tc.tile_pool(name="hnew", bufs=3) as hpool,
            tc.tile_pool(name="pre", bufs=2, space="PSUM") as prepool,
            tc.tile_pool(name="trp", bufs=2, space="PSUM") as trpool,
            tc.tile_pool(name="agio", bufs=2, space="DRAM") as dpool,
        ):
            # --- constants, loaded once -------------------------------
            wx_sb = cpool.tile([128, DK, PW], f32, name="wx_sb")
            nc.sync.dma_start(out=wx_sb[:], in_=wx_sl.ap().rearrange("c p n -> p c n"))
            wh_sb = cpool.tile([128, UK, PW], f32, name="wh_sb")
            nc.sync.dma_start(out=wh_sb[:], in_=wh_sl.ap().rearrange("c p n -> p c n"))
            b_sb = cpool.tile([1, PW], f32, name="b_sb")
            nc.sync.dma_start(out=b_sb[:], in_=b_sl[:])
            ones_sb = cpool.tile([1, 128], f32, name="ones_sb")
            nc.sync.dma_start(out=ones_sb[:], in_=ones[:])
            tau_sb = cpool.tile([B, SL], f32, name="tau_sb")
            nc.sync.dma_start(out=tau_sb[:], in_=tau_b[:])
            ndt_sb = cpool.tile([B, n_steps], f32, name="ndt_sb")
            nc.sync.dma_start(out=ndt_sb[:], in_=ndt[:])
            ident_sb = cpool.tile([128, 128], f32, name="ident_sb")
            nc.sync.dma_start(out=ident_sb[:], in_=ident[:])
            ones_bc = cpool.tile([B, SL], f32, name="ones_bc")
            nc.vector.memset(ones_bc[:], 1.0)

            # initial state
            h_prev = hpool.tile([B, SL], f32, name="h_new")
            nc.sync.dma_start(out=h_prev[:], in_=h0_sl[:])
            hT0 = hTpool.tile([128, UK, B], f32, name="hTall")
            nc.sync.dma_start(out=hT0[:], in_=h0T.ap().rearrange("c p b -> p c b"))
            hT_cur = [hT0[:, j, :] for j in range(UK)]

            # --- the recurrence ---------------------------------------
            for tv in range(n_steps * repeat):
                t = tv % n_steps
                # x_t^T chunks: [d-chunk partitions, batch free]
                xt = xpool.tile([128, DK, B], f32, name="xt")
                nc.sync.dma_start(out=xt[:], in_=xT[t].rearrange("c p b -> p c b"))

                # split PSUM accumulation into f-half and a-half: pre_f stops
                # 8 matmuls earlier, so the f-chain (tanh -> g -> exp) hides
                # under the a-half h-matmuls.
                pre_f = prepool.tile([B, SL], f32, name="pre_f")
                pre_a = prepool.tile([B, SL], f32, name="pre_a")
                # bias + x part: no dependency on h -> runs during the
                # previous step's AllGather wait.
                nc.tensor.matmul(pre_f[:], mm(ones_sb[:]), mm(b_sb[:, 0:SL]),
                                 start=True, stop=False)
                nc.tensor.matmul(pre_a[:], mm(ones_sb[:]), mm(b_sb[:, SL:PW]),
                                 start=True, stop=False)
                for c in range(DK):
                    nc.tensor.matmul(
                        pre_f[:], mm(xt[:, c, :]), mm(wx_sb[:, c, 0:SL]),
                        start=False, stop=False,
                    )
                    nc.tensor.matmul(
                        pre_a[:], mm(xt[:, c, :]), mm(wx_sb[:, c, SL:PW]),
                        start=False, stop=False,
                    )
                # h part: all f-half chunks first (early stop), then a-half
                for j in range(UK):
                    nc.tensor.matmul(
                        pre_f[:],
                        mm(hT_cur[j]),
                        mm(wh_sb[:, j, 0:SL]),
                        start=False,
                        stop=(j == UK - 1),
                    )
                for j in range(UK):
                    nc.tensor.matmul(
                        pre_a[:],
                        mm(hT_cur[j]),
                        mm(wh_sb[:, j, SL:PW]),
                        start=False,
                        stop=(j == UK - 1),
                    )

                # sigmoid via tanh keeps Tanh+Exp in ONE activation table
                # set (exp_and_others): no per-step ACT_TABLE_LOAD thrash.
                # sigmoid(x) = 0.5*tanh(x/2) + 0.5; host ships tau+0.5, so
                # g = (tau+0.5) + 0.5*tanh(pre_f/2) = tau + sigmoid(pre_f).
                # h' = h*d + a*(1-d): the f-chain (tanh_f -> g -> exp) plus
                # h*d and (1-d) depend only on pre_f, so they all hide under
                # the a-half matmuls; after tanh_a only two DVE ops remain.
                # ACT queue order: tanh_f, exp, tanh_a (exp must not sit
                # behind tanh_a, which waits for the full a-half).
                f = apool.tile([B, SL], f32, name="f")
                nc.scalar.activation(f[:], pre_f[:], AF.Tanh, scale=0.5)
                g = apool.tile([B, SL], f32, name="g")
                nc.vector.scalar_tensor_tensor(
                    g[:], f[:], 0.5, tau_sb[:],
                    op0=mybir.AluOpType.mult, op1=mybir.AluOpType.add,
                )
                dcy = apool.tile([B, SL], f32, name="dcy")
                nc.scalar.activation(
                    dcy[:], g[:], AF.Exp, scale=ndt_sb[:, t : t + 1]
                )
                omd = apool.tile([B, SL], f32, name="omd")
                nc.vector.scalar_tensor_tensor(
                    omd[:], dcy[:], -1.0, ones_bc[:],
                    op0=mybir.AluOpType.mult, op1=mybir.AluOpType.add,
                )
                t2 = apool.tile([B, SL], f32, name="t2")
                nc.vector.tensor_mul(t2[:], h_prev[:], dcy[:])
                a = apool.tile([B, SL], f32, name="a")
                nc.scalar.activation(a[:], pre_a[:], AF.Tanh)
                t1 = apool.tile([B, SL], f32, name="t1")
                nc.vector.tensor_mul(t1[:], a[:], omd[:])
                h_new = hpool.tile([B, SL], f32, name="h_new")
                nc.vector.tensor_add(h_new[:], t1[:], t2[:])

                nc.sync.dma_start(out=ys_sl[t], in_=h_new[:])

                if tv == n_steps * repeat - 1:
                    h_prev = h_new
                    break

                # h'^T slice for the next step's matmul
                trp = trpool.tile([128, B], f32, name="trp")
                nc.tensor.transpose(trp[:], h_new[:], ident_sb[:])
                trs = apool.tile([128, B], f32, name="trs")
                nc.vector.tensor_copy(trs[:], trp[:])
                ag_in = dpool.tile([128, B], f32, name="ag_in")
                nc.sync.dma_start(out=ag_in[:], in_=trs[:])
                if use_collective:
                    ag_out = dpool.tile(
                        [UK * 128, B], f32, name="ag_out", addr_space="Shared"
                    )
                    nc.gpsimd.collective_compute(
                        "AllGather",
                        mybir.AluOpType.bypass,
                        replica_groups=RG,
                        ins=[ag_in[:].opt()],
                        outs=[ag_out[:].opt()],
                    )
                hTall = hTpool.tile([128, UK, B], f32, name="hTall")
                if use_collective:
                    # one strided DMA gathers all 8 K-chunks: [(c p) b -> p c b]
                    nc.sync.dma_start(
                        out=hTall[:],
                        in_=ag_out.rearrange("(c p) b -> p c b", p=128),
                    )
                else:
                    # timing-only bisect variant: local slice in place of
                    # the gathered one (numerically wrong on purpose)
                    for j in range(UK):
                        nc.sync.dma_start(out=hTall[:, j, :], in_=ag_in[:])
                hT_cur = [hTall[:, j, :] for j in range(UK)]
                h_prev = h_new

    nc.compile()
    return nc


def _prep_inputs(features, time_steps, Wx, Wh, b, w_tau, h0, n_steps):
    """Host-side sharding + layout transforms -> per-core in_maps."""
    f32 = np.float32
    features = np.asarray(features, dtype=f32)
    time_steps = np.asarray(time_steps, dtype=f32)
    Wx = np.asarray(Wx, dtype=f32)
    Wh = np.asarray(Wh, dtype=f32)
    b = np.asarray(b, dtype=f32)
    w_tau = np.asarray(w_tau, dtype=f32)
    h0 = np.asarray(h0, dtype=f32)

    # softplus(w_tau) + 0.5 (the 0.5 folds the tanh-form sigmoid offset)
    tau = (np.log1p(np.exp(w_tau)) + 0.5).astype(f32)

    xT = np.ascontiguousarray(features.transpose(1, 2, 0)).reshape(n_steps, DK, 128, B)
    ndt = np.ascontiguousarray(-time_steps)                      # [B, T]
    h0T = np.ascontiguousarray(h0.T).reshape(UK, 128, B)
    ones = np.ones((1, 128), dtype=f32)
    ident = np.eye(128, dtype=f32)

    in_maps = []
    for k in range(NC):
        cols = np.concatenate(
            [np.arange(k * SL, (k + 1) * SL), U + np.arange(k * SL, (k + 1) * SL)]
        )
        in_maps.append(
            {
                "xT": xT,
                "wx_sl": np.ascontiguousarray(Wx[:, cols]).reshape(DK, 128, PW),
                "wh_sl": np.ascontiguousarray(Wh[:, cols]).reshape(UK, 128, PW),
                "b_sl": np.ascontiguousarray(b[cols]).reshape(1, PW),
                "tau_b": np.ascontiguousarray(
                    np.broadcast_to(tau[k * SL : (k + 1) * SL], (B, SL))
                ),
                "ndt": ndt,
                "h0T": h0T,
                "h0_sl": np.ascontiguousarray(h0[:, k * SL : (k + 1) * SL]),
                "ones": ones,
                "ident": ident,
            }
        )
    return in_maps


def _assemble(results):
    """[T, B, SL] slices per core -> [B, T, U] full output."""
    ys = np.concatenate([r["ys_sl"] for r in results], axis=2)  # [T, B, U]
    return np.ascontiguousarray(ys.transpose(1, 0, 2))


def kernel(features, time_steps, Wx, Wh, b, w_tau, h0, _trace=False):
    from concourse import bass_utils

    n_steps = features.shape[1]
    if n_steps not in _CACHE:
        _CACHE[n_steps] = _build(n_steps)
    nc = _CACHE[n_steps]

    in_maps = _prep_inputs(features, time_steps, Wx, Wh, b, w_tau, h0, n_steps)
    try:
        res = bass_utils.run_bass_kernel_spmd(
            nc, in_maps, core_ids=list(range(NC)), trace=_trace
        )
    except ModuleNotFoundError:
        # no NTFF profiling hook in this container — run untraced
        res = bass_utils.run_bass_kernel_spmd(
            nc, in_maps, core_ids=list(range(NC)), trace=False
        )
    out = _assemble(res.results)
    if _trace:
        return out, res
    return out


if __name__ == "__main__":
    # smoke test with random data
    rng = np.random.default_rng(0)
    feats = rng.standard_normal((B, T, D), dtype=np.float32)
    ts = rng.random((B, T), dtype=np.float32)
    Wx = rng.standard_normal((D, 2 * U), dtype=np.float32) / np.sqrt(D)
    Wh = rng.standard_normal((U, 2 * U), dtype=np.float32) / np.sqrt(U)
    b = np.zeros((2 * U,), dtype=np.float32)
    w_tau = rng.random((U,), dtype=np.float32)
    h0 = np.zeros((B, U), dtype=np.float32)
    out = kernel(feats, ts, Wx, Wh, b, w_tau, h0)
    print("output", out.shape, out.dtype)

